# revision 3
# baseline (speedup 1.0000x reference)
"""Segment-max (GridPooling) kernel v2.2 for 8 trn2 NeuronCores.

v2.1 -> v2.2:
  * Output + mask DMAs move to the Activation engine's HWDGE queue so they
    overlap with input DMAs on the SP queue instead of serializing.
  * Variable-size last tile (multiple of 64 slots) removes the ~2.3%
    round-up-to-TF padding of each chunk stream.
  * Optional removal of the per-tile DVE drain (DRAIN flag).
"""
import sys

if "/opt/trn_rl_repo" not in sys.path:
    sys.path.insert(0, "/opt/trn_rl_repo")

import os

import numpy as np
import ml_dtypes

W = 8
TF = 16384          # max slots per tile
NCORES = 8
NEG = np.float32(-1e30)
BF16 = ml_dtypes.bfloat16
DRAIN = os.environ.get("K_DRAIN", "1") == "1"

_nc_cache = {}


def _tile_sizes(l_half):
    """Split l_half (multiple of 64) into tiles of TF with a short last tile."""
    sizes = [TF] * (l_half // TF)
    rem = l_half % TF
    if rem:
        sizes.append(rem)
    return sizes


def _build_nc(tf_sizes, reps=1):
    import contextlib
    import concourse.bass as bass
    from concourse import mybir

    ntiles = len(tf_sizes)
    assert ntiles <= 32, "mask-packing trick requires ntiles <= 32"
    GT = TF // W
    gt_sizes = [t // W for t in tf_sizes]
    bf16 = mybir.dt.bfloat16
    nc = bass.Bass()
    x_ext = nc.declare_dram_parameter("x", [ntiles * 128, TF], bf16, isOutput=False)
    m_ext = nc.declare_dram_parameter("m", [128, GT], bf16, isOutput=False)
    s_ext = nc.declare_dram_parameter("scan", [ntiles * 128, GT], bf16, isOutput=True)

    NB = 3

    ctx = contextlib.ExitStack()
    with ctx:
        xt = [ctx.enter_context(nc.sbuf_tensor(f"xt{i}", [128, TF], bf16)) for i in range(NB)]
        mp = ctx.enter_context(nc.sbuf_tensor("mp", [128, GT], bf16))
        bc = [ctx.enter_context(nc.sbuf_tensor(f"bc{i}", [128, GT], bf16)) for i in range(2)]
        f1 = ctx.enter_context(nc.sbuf_tensor("f1", [128, TF // 2], bf16))
        f2 = ctx.enter_context(nc.sbuf_tensor("f2", [128, TF // 4], bf16))
        red = [ctx.enter_context(nc.sbuf_tensor(f"red{i}", [128, GT], bf16)) for i in range(2)]
        st = [ctx.enter_context(nc.sbuf_tensor(f"st{i}", [128, GT], bf16)) for i in range(2)]
        in_sems = [ctx.enter_context(nc.semaphore(f"in_sem{i}")) for i in range(NB)]
        mk_sem = ctx.enter_context(nc.semaphore("mk_sem"))
        out_sems = [ctx.enter_context(nc.semaphore(f"out_sem{i}")) for i in range(2)]
        v_sem = ctx.enter_context(nc.semaphore("v_sem"))
        block = ctx.enter_context(nc.Block())

        total = ntiles * reps

        def in_dma(s, j):
            jd = j % ntiles
            s.dma_start(xt[j % NB][:, 0:tf_sizes[jd]],
                        x_ext[128 * jd:128 * (jd + 1), 0:tf_sizes[jd]]
                        ).then_inc(in_sems[j % NB], 16)

        @block.sync
        def _(s):
            s.dma_start(mp[:], m_ext[:]).then_inc(mk_sem, 16)
            for i in range(min(NB, total)):
                in_dma(s, i)
            for i in range(total):
                d = i % ntiles
                s.wait_ge(v_sem, i + 1)
                s.dma_start(s_ext[128 * d:128 * (d + 1), 0:gt_sizes[d]],
                            st[i % 2][:, 0:gt_sizes[d]]).then_inc(out_sems[i % 2], 16)
                if i + NB < total:
                    in_dma(s, i + NB)

        @block.vector
        def _(v):
            mx = mybir.AluOpType.max
            v.wait_ge(mk_sem, 16)
            for i in range(total):
                d = i % ntiles
                tf = tf_sizes[d]
                gt = gt_sizes[d]
                h = tf // 2
                v.stream_shuffle(bc[i % 2][:, 0:gt], mp[:, 0:gt], mask=[d] * 32)
                v.wait_ge(in_sems[i % NB], 16 * (i // NB + 1))
                x = xt[i % NB]
                v.tensor_tensor(f1[:, 0:h], x[:, 0:h], x[:, h:tf], mx)
                v.tensor_tensor(f2[:, 0:h // 2], f1[:, 0:h // 2], f1[:, h // 2:h], mx)
                v.tensor_tensor(red[i % 2][:, 0:gt], f2[:, 0:gt], f2[:, gt:2 * gt], mx)
                if i >= 2:
                    v.wait_ge(out_sems[i % 2], 16 * (i // 2))
                if DRAIN:
                    v.drain()
                if i == 0:
                    init = float(NEG)
                else:
                    pgt = gt_sizes[(i - 1) % ntiles]
                    init = st[(i - 1) % 2][:, pgt - 1:pgt]
                v.tensor_tensor_scan(
                    st[i % 2][:, 0:gt], bc[i % 2][:, 0:gt], red[i % 2][:, 0:gt],
                    initial=init,
                    op0=mybir.AluOpType.add, op1=mybir.AluOpType.max,
                ).then_inc(v_sem, 1)

    return nc


def _preprocess(sig, idx, S):
    """Sort+pad on host; build tile-major bf16 device arrays."""
    N, D = sig.shape
    assert D == 64, f"kernel assumes D=64, got {D}"
    counts = np.bincount(idx, minlength=S)
    order = np.argsort(idx, kind="stable")
    pc = ((counts + W - 1) // W) * W
    padded_starts = np.zeros(S + 1, np.int64)
    np.cumsum(pc, out=padded_starts[1:])
    L = int(padded_starts[-1])
    cstart = np.zeros(S + 1, np.int64)
    np.cumsum(counts, out=cstart[1:])

    sid = np.repeat(np.arange(S, dtype=np.int64), pc)
    pos = np.arange(L, dtype=np.int64) - padded_starts[sid]
    src_sorted = cstart[sid] + np.minimum(pos, counts[sid] - 1)
    perm = order[src_sorted]                  # padded stream -> signal row

    targets = (L * np.arange(1, 16, dtype=np.int64)) // 16
    split_segs = np.searchsorted(padded_starts, targets, side="left")
    seg_bounds = np.concatenate([[0], split_segs, [S]])
    seg_bounds = np.maximum.accumulate(seg_bounds)
    slot_bounds = padded_starts[seg_bounds]

    lh_real = np.diff(slot_bounds)
    l_half = int(-(-int(lh_real.max()) // 64) * 64)
    tf_sizes = _tile_sizes(l_half)
    ntiles = len(tf_sizes)
    GT = TF // W
    starts = np.concatenate([[0], np.cumsum(tf_sizes)])

    sig_t = np.ascontiguousarray(sig.T.astype(BF16))   # [64, N] bf16
    in_maps = []
    plans = []   # per half: (core, rows_lo, s_lo, s_hi, base_slot)
    for c in range(NCORES):
        X = np.zeros((ntiles, 128, TF), BF16)
        M = np.zeros((128, GT), np.float32)
        for h in range(2):
            k = 2 * c + h
            s_lo, s_hi = int(seg_bounds[k]), int(seg_bounds[k + 1])
            b0, b1 = int(slot_bounds[k]), int(slot_bounds[k + 1])
            hperm = perm[b0:b1]
            if len(hperm) < l_half:
                pad_src = hperm[-1] if len(hperm) else 0
                hperm = np.concatenate(
                    [hperm, np.full(l_half - len(hperm), pad_src, np.int64)])
            arr = sig_t[:, hperm]                       # [64, l_half]
            mrow = np.zeros(l_half // W, np.float32)
            starts_local = (padded_starts[s_lo:s_hi] - b0) // W
            mrow[starts_local[(pc[s_lo:s_hi] > 0)]] = NEG
            mrow[(b1 - b0) // W:] = NEG       # dummy tail groups: isolate
            for t, tf in enumerate(tf_sizes):
                gt = tf // W
                a = arr[:, starts[t]:starts[t + 1]]
                # slot s = g*W + w (within tile) -> X[t, f, w*gt + g]
                X[t, 64 * h:64 * (h + 1), 0:tf] = (
                    a.reshape(64, gt, W).transpose(0, 2, 1).reshape(64, tf))
                mt = mrow[starts[t] // W:starts[t + 1] // W]
                M[64 * h + t, 0:gt] = mt
                M[64 * h + 32 + t, 0:gt] = mt
            plans.append((c, 64 * h, s_lo, s_hi, b0))
        in_maps.append({"x": X.reshape(ntiles * 128, TF),
                        "m": M.astype(BF16)})
    return in_maps, plans, padded_starts, pc, tuple(tf_sizes)


def kernel(signal, cell_idx, num_segments):
    from concourse.bass_utils import run_bass_kernel_spmd

    sig = np.asarray(signal, dtype=np.float32)
    idx = np.asarray(cell_idx).astype(np.int64)
    S = int(num_segments)

    in_maps, plans, padded_starts, pc, tf_sizes = _preprocess(sig, idx, S)

    if tf_sizes not in _nc_cache:
        _nc_cache[tf_sizes] = _build_nc(tf_sizes)
    nc = _nc_cache[tf_sizes]

    res = run_bass_kernel_spmd(nc, in_maps, core_ids=list(range(NCORES)))

    ntiles = len(tf_sizes)
    GT = TF // W
    gt_sizes = [t // W for t in tf_sizes]
    out = np.full((S, sig.shape[1]), -np.inf, np.float32)
    for (c, r0, s_lo, s_hi, b0) in plans:
        if s_hi <= s_lo:
            continue
        scan = np.asarray(res.results[c]["scan"]).reshape(ntiles, 128, GT)
        cols = [scan[t, r0:r0 + 64, 0:gt_sizes[t]] for t in range(ntiles)]
        scan_half = np.concatenate(cols, axis=1).astype(np.float32)  # [64, g_half]
        nz = pc[s_lo:s_hi] > 0
        ends_local = (padded_starts[s_lo + 1:s_hi + 1] - b0) // W - 1
        out[np.arange(s_lo, s_hi)[nz]] = scan_half[:, ends_local[nz]].T
    return out
